# revision 4
# baseline (speedup 1.0000x reference)
"""Trainium2 Bass kernel for nn_KVCacheQuantizer: int4 group-wise KV-cache
quantize + dequantize round trip.

Full inputs k, v: [4, 32, 4096, 128] fp32. Outputs:
  k_q  [4,32,4096,4,32] int8, k_scale [4,32,4096,4] f32, same for v,
  k_dq / v_dq [4,32,4096,4,32] f32.

Sharded data-parallel over the 128 (batch, head) slabs: core c takes 16
slabs of k and 16 of v; group-wise math (groups of 32 along head_dim) is
fully local.

Per-core pipeline per [128, 2048] fp32 tile (64 groups of 32 per partition):
  a  = absmax over each group          (reduce, abs)
  s  = max(a, 1e-8) * (1/7)
  r  = 1/s                             (DVE iterative reciprocal)
  q  = x * r                           (broadcast per group)
  n  = rne(q)  via +/- 1.5*2^23 magic
  i8 = int8(n)
  dq = n * s
This matches the XLA-on-neuron lowering of the jax reference bit for bit
(verified: the neuron backend lowers x/s to x*reciprocal(s) the same way).
"""

import numpy as np

import concourse.bacc as bacc
import concourse.tile as tile
import concourse.mybir as mybir
from concourse.bass_utils import run_bass_kernel_spmd

B, H, S, D = 4, 32, 4096, 128
GS = 32                      # quantization group size
NCORES = 8
SLABS_PER_CORE = (B * H) // NCORES          # 16 slabs of each tensor
GROUPS_PER_SLAB = S * D // GS               # 16384
T = 64                       # groups per partition per tile
FD = T * GS                  # 2048 fp32 free elems per partition
HALVES = GROUPS_PER_SLAB // (128 * T)       # 2 tiles per slab

EPS = 1e-8
INV7 = float(np.float32(1.0) / np.float32(7.0))
MAGIC = 12582912.0           # 1.5 * 2^23: (q + MAGIC) - MAGIC == rne(q)

# engine knobs (tunable)
REDUCE_ENGINE = "vector"     # free-axis reduce is DVE-only
CAST_ENGINE = "gpsimd"       # "gpsimd" | "vector" | "scalar"
ROUND_ENGINE = "scalar"      # "scalar" (2 ACT ops) | "vector" (1 TS op)
DQ_ENGINE = "gpsimd"         # "gpsimd" | "vector"
BUFS = 3

_nc_cache = {}


def _build():
    nc = bacc.Bacc("TRN2", target_bir_lowering=False, debug=False,
                   num_devices=NCORES)
    f32, i8t = mybir.dt.float32, mybir.dt.int8
    n_slabs = SLABS_PER_CORE

    ins = {}
    outs = {}
    for name in ("k", "v"):
        ins[name] = nc.dram_tensor(name, [n_slabs, S, D], f32,
                                   kind="ExternalInput").ap()
        outs[name + "q"] = nc.dram_tensor(name + "q",
                                          [n_slabs, S, D // GS, GS], i8t,
                                          kind="ExternalOutput").ap()
        outs[name + "s"] = nc.dram_tensor(name + "s",
                                          [n_slabs, S, D // GS], f32,
                                          kind="ExternalOutput").ap()
        outs[name + "dq"] = nc.dram_tensor(name + "dq",
                                           [n_slabs, S, D // GS, GS], f32,
                                           kind="ExternalOutput").ap()

    # flat per-(slab, half) tile views: [n_slabs, HALVES, 128, FD]
    def tiled(ap, elems_per_part):
        flat = ap.rearrange("j a b c -> j (a b c)") if ap.ndim == 4 else \
               ap.rearrange("j a b -> j (a b)")
        return flat.rearrange("j (h p f) -> j h p f", h=HALVES, p=128)

    with tile.TileContext(nc) as tc:
        with (
            tc.tile_pool(name="xp", bufs=BUFS + 1) as xp,
            tc.tile_pool(name="stats", bufs=BUFS + 1) as statsp,
            tc.tile_pool(name="work", bufs=BUFS) as workp,
            tc.tile_pool(name="outp", bufs=BUFS) as outp,
        ):
            for name in ("k", "v"):
                x_t = tiled(ins[name], FD)
                q_t = tiled(outs[name + "q"], FD)
                s_t = tiled(outs[name + "s"], T)
                dq_t = tiled(outs[name + "dq"], FD)
                for j in range(SLABS_PER_CORE):
                    for h in range(HALVES):
                        _tile_body(nc, xp, statsp, workp, outp,
                                   x_t[j, h], q_t[j, h], s_t[j, h],
                                   dq_t[j, h])
    nc.compile()
    return nc


def _tile_body(nc, xp, statsp, workp, outp, x_in, q_out, s_out, dq_out):
    f32, i8t = mybir.dt.float32, mybir.dt.int8

    x = xp.tile([128, FD], f32, tag="x")
    nc.sync.dma_start(x[:], x_in)
    xg = x[:].rearrange("p (t e) -> p t e", t=T)

    a = statsp.tile([128, T], f32, tag="a")
    red_eng = nc.gpsimd if REDUCE_ENGINE == "gpsimd" else nc.vector
    red_eng.tensor_reduce(a[:], xg, op=mybir.AluOpType.max,
                          axis=mybir.AxisListType.X,
                          apply_absolute_value=True)

    s = statsp.tile([128, T], f32, tag="s")
    nc.vector.tensor_scalar(s[:], a[:], float(EPS), INV7,
                            op0=mybir.AluOpType.max,
                            op1=mybir.AluOpType.mult)
    nc.sync.dma_start(s_out, s[:])

    r = statsp.tile([128, T], f32, tag="r")
    nc.vector.reciprocal(r[:], s[:])

    q = workp.tile([128, FD], f32, tag="q")
    qg = q[:].rearrange("p (t e) -> p t e", t=T)
    nc.vector.tensor_tensor(qg, xg, r[:].to_broadcast((128, T, GS)),
                            op=mybir.AluOpType.mult)

    n = workp.tile([128, FD], f32, tag="n")
    if ROUND_ENGINE == "scalar":
        t1 = workp.tile([128, FD], f32, tag="t1")
        nc.scalar.activation(t1[:], q[:], mybir.ActivationFunctionType.Copy,
                             bias=MAGIC)
        nc.scalar.activation(n[:], t1[:], mybir.ActivationFunctionType.Copy,
                             bias=-MAGIC)
    else:
        nc.vector.tensor_scalar(n[:], q[:], MAGIC, MAGIC,
                                op0=mybir.AluOpType.add,
                                op1=mybir.AluOpType.subtract)

    i8 = outp.tile([128, FD], i8t, tag="i8")
    cast_eng = {"gpsimd": nc.gpsimd, "vector": nc.vector,
                "scalar": nc.scalar}[CAST_ENGINE]
    if CAST_ENGINE == "scalar":
        cast_eng.activation(i8[:], n[:], mybir.ActivationFunctionType.Copy)
    else:
        cast_eng.tensor_copy(i8[:], n[:])
    nc.sync.dma_start(q_out, i8[:])

    dq = outp.tile([128, FD], f32, tag="dq")
    dqg = dq[:].rearrange("p (t e) -> p t e", t=T)
    dq_eng = nc.gpsimd if DQ_ENGINE == "gpsimd" else nc.vector
    dq_eng.tensor_tensor(dqg, n[:].rearrange("p (t e) -> p t e", t=T),
                         s[:].to_broadcast((128, T, GS)),
                         op=mybir.AluOpType.mult)
    nc.sync.dma_start(dq_out, dq[:])


def _get_nc():
    key = (REDUCE_ENGINE, CAST_ENGINE, ROUND_ENGINE, DQ_ENGINE, BUFS, T)
    if key not in _nc_cache:
        _nc_cache[key] = _build()
    return _nc_cache[key]


def kernel(k: np.ndarray, v: np.ndarray, _trace: bool = False):
    nc = _get_nc()
    ksl = np.ascontiguousarray(k).reshape(B * H, S, D)
    vsl = np.ascontiguousarray(v).reshape(B * H, S, D)
    in_maps = []
    for c in range(NCORES):
        sl = slice(c * SLABS_PER_CORE, (c + 1) * SLABS_PER_CORE)
        in_maps.append({"k": ksl[sl], "v": vsl[sl]})
    res = run_bass_kernel_spmd(nc, in_maps, list(range(NCORES)),
                               trace=_trace)
    kernel._last_results = res

    def gather(oname, dtype, tail):
        parts = [res.results[c][oname] for c in range(NCORES)]
        return np.concatenate(parts, axis=0).reshape(B, H, S, *tail) \
                 .astype(dtype, copy=False)

    k_q = gather("kq", np.int8, (D // GS, GS))
    k_scale = gather("ks", np.float32, (D // GS,))
    k_dq = gather("kdq", np.float32, (D // GS, GS))
    v_q = gather("vq", np.int8, (D // GS, GS))
    v_scale = gather("vs", np.float32, (D // GS,))
    v_dq = gather("vdq", np.float32, (D // GS, GS))
    return k_q, k_scale, v_q, v_scale, k_dq, v_dq


# revision 7
# speedup vs baseline: 1.4139x; 1.4139x over previous
"""Trainium2 Bass kernel for nn_KVCacheQuantizer: int4 group-wise KV-cache
quantize + dequantize round trip.

Full inputs k, v: [4, 32, 4096, 128] fp32. Outputs:
  k_q  [4,32,4096,4,32] int8, k_scale [4,32,4096,4] f32, same for v,
  k_dq / v_dq [4,32,4096,4,32] f32.

Sharded data-parallel over the 128 (batch, head) slabs: core c takes 16
slabs of k and 16 of v; group-wise math (groups of 32 along head_dim) is
fully local.

Per-core pipeline per [128, 2048] fp32 tile (64 groups of 32 per partition):
  a  = absmax over each group          (reduce, abs)
  s  = max(a, 1e-8) * (1/7)
  r  = 1/s                             (DVE iterative reciprocal)
  q  = x * r                           (broadcast per group)
  n  = rne(q)  via +/- 1.5*2^23 magic
  i8 = int8(n)
  dq = n * s
This matches the XLA-on-neuron lowering of the jax reference bit for bit
(verified: the neuron backend lowers x/s to x*reciprocal(s) the same way).
"""

import numpy as np

import concourse.bacc as bacc
import concourse.tile as tile
import concourse.mybir as mybir
from concourse.bass_utils import run_bass_kernel_spmd

B, H, S, D = 4, 32, 4096, 128
GS = 32                      # quantization group size
NCORES = 8
SLABS_PER_CORE = (B * H) // NCORES          # 16 slabs of each tensor
GROUPS_PER_SLAB = S * D // GS               # 16384
T = 64                       # groups per partition per tile
FD = T * GS                  # 2048 fp32 free elems per partition
HALVES = GROUPS_PER_SLAB // (128 * T)       # 2 tiles per slab

EPS = 1e-8
INV7 = float(np.float32(1.0) / np.float32(7.0))
MAGIC = 12582912.0           # 1.5 * 2^23: (q + MAGIC) - MAGIC == rne(q)

# engine knobs (tunable)
FUSE_CAST = True             # fuse round step 2 + int8 cast into one ACT op
DQ_FROM_I8 = True            # dq = int8_tile * s (mixed dtype TT)
CAST_ENGINE = "scalar"       # used when FUSE_CAST is False
DQ_ENGINE = "gpsimd"         # "gpsimd" | "vector"
SCALE_ENGINE = "vector"      # "vector" | "gpsimd"
BUFS = 3

_nc_cache = {}


def _build():
    nc = bacc.Bacc("TRN2", target_bir_lowering=False, debug=False,
                   num_devices=NCORES)
    f32, i8t = mybir.dt.float32, mybir.dt.int8
    n_slabs = SLABS_PER_CORE

    ins = {}
    outs = {}
    for name in ("k", "v"):
        ins[name] = nc.dram_tensor(name, [n_slabs, S, D], f32,
                                   kind="ExternalInput").ap()
        outs[name + "q"] = nc.dram_tensor(name + "q",
                                          [n_slabs, S, D // GS, GS], i8t,
                                          kind="ExternalOutput").ap()
        outs[name + "s"] = nc.dram_tensor(name + "s",
                                          [n_slabs, S, D // GS], f32,
                                          kind="ExternalOutput").ap()
        outs[name + "dq"] = nc.dram_tensor(name + "dq",
                                           [n_slabs, S, D // GS, GS], f32,
                                           kind="ExternalOutput").ap()

    # flat per-(slab, half) tile views: [n_slabs, HALVES, 128, FD]
    def tiled(ap, elems_per_part):
        flat = ap.rearrange("j a b c -> j (a b c)") if ap.ndim == 4 else \
               ap.rearrange("j a b -> j (a b)")
        return flat.rearrange("j (h p f) -> j h p f", h=HALVES, p=128)

    with tile.TileContext(nc) as tc:
        with (
            tc.tile_pool(name="xp", bufs=BUFS + 1) as xp,
            tc.tile_pool(name="stats", bufs=BUFS + 1) as statsp,
            tc.tile_pool(name="work", bufs=BUFS) as workp,
            tc.tile_pool(name="outp", bufs=BUFS) as outp,
        ):
            for name in ("k", "v"):
                x_t = tiled(ins[name], FD)
                q_t = tiled(outs[name + "q"], FD)
                s_t = tiled(outs[name + "s"], T)
                dq_t = tiled(outs[name + "dq"], FD)
                for j in range(SLABS_PER_CORE):
                    for h in range(HALVES):
                        _tile_body(nc, xp, statsp, workp, outp,
                                   x_t[j, h], q_t[j, h], s_t[j, h],
                                   dq_t[j, h])
    nc.compile()
    return nc


def _tile_body(nc, xp, statsp, workp, outp, x_in, q_out, s_out, dq_out):
    f32, i8t = mybir.dt.float32, mybir.dt.int8

    x = xp.tile([128, FD], f32, tag="x")
    nc.sync.dma_start(x[:], x_in)
    xg = x[:].rearrange("p (t e) -> p t e", t=T)

    a = statsp.tile([128, T], f32, tag="a")
    nc.vector.tensor_reduce(a[:], xg, op=mybir.AluOpType.max,
                            axis=mybir.AxisListType.X,
                            apply_absolute_value=True)

    s = statsp.tile([128, T], f32, tag="s")
    scale_eng = nc.gpsimd if SCALE_ENGINE == "gpsimd" else nc.vector
    scale_eng.tensor_scalar(s[:], a[:], float(EPS), INV7,
                            op0=mybir.AluOpType.max,
                            op1=mybir.AluOpType.mult)
    nc.sync.dma_start(s_out, s[:])

    r = statsp.tile([128, T], f32, tag="r")
    nc.vector.reciprocal(r[:], s[:])

    q = workp.tile([128, FD], f32, tag="q")
    qg = q[:].rearrange("p (t e) -> p t e", t=T)
    nc.vector.tensor_tensor(qg, xg, r[:].to_broadcast((128, T, GS)),
                            op=mybir.AluOpType.mult)

    t1 = workp.tile([128, FD], f32, tag="t1")
    nc.scalar.activation(t1[:], q[:], mybir.ActivationFunctionType.Copy,
                         bias=MAGIC)

    i8 = outp.tile([128, FD], i8t, tag="i8")
    if FUSE_CAST:
        nc.scalar.activation(i8[:], t1[:], mybir.ActivationFunctionType.Copy,
                             bias=-MAGIC)
        n_ap = i8
    else:
        n = workp.tile([128, FD], f32, tag="n")
        nc.scalar.activation(n[:], t1[:], mybir.ActivationFunctionType.Copy,
                             bias=-MAGIC)
        cast_eng = {"gpsimd": nc.gpsimd, "vector": nc.vector,
                    "scalar": nc.scalar}[CAST_ENGINE]
        if CAST_ENGINE == "scalar":
            cast_eng.activation(i8[:], n[:],
                                mybir.ActivationFunctionType.Copy)
        else:
            cast_eng.tensor_copy(i8[:], n[:])
        n_ap = n
    nc.sync.dma_start(q_out, i8[:])

    dq_src = i8 if DQ_FROM_I8 else n_ap
    dq = outp.tile([128, FD], f32, tag="dq")
    dqg = dq[:].rearrange("p (t e) -> p t e", t=T)
    dq_eng = nc.gpsimd if DQ_ENGINE == "gpsimd" else nc.vector
    dq_eng.tensor_tensor(dqg, dq_src[:].rearrange("p (t e) -> p t e", t=T),
                         s[:].to_broadcast((128, T, GS)),
                         op=mybir.AluOpType.mult)
    nc.sync.dma_start(dq_out, dq[:])


def _get_nc():
    key = (FUSE_CAST, DQ_FROM_I8, CAST_ENGINE, DQ_ENGINE, SCALE_ENGINE,
           BUFS, T)
    if key not in _nc_cache:
        _nc_cache[key] = _build()
    return _nc_cache[key]


def kernel(k: np.ndarray, v: np.ndarray, _trace: bool = False):
    nc = _get_nc()
    ksl = np.ascontiguousarray(k).reshape(B * H, S, D)
    vsl = np.ascontiguousarray(v).reshape(B * H, S, D)
    in_maps = []
    for c in range(NCORES):
        sl = slice(c * SLABS_PER_CORE, (c + 1) * SLABS_PER_CORE)
        in_maps.append({"k": ksl[sl], "v": vsl[sl]})
    res = run_bass_kernel_spmd(nc, in_maps, list(range(NCORES)),
                               trace=_trace)
    kernel._last_results = res

    def gather(oname, dtype, tail):
        parts = [res.results[c][oname] for c in range(NCORES)]
        return np.concatenate(parts, axis=0).reshape(B, H, S, *tail) \
                 .astype(dtype, copy=False)

    k_q = gather("kq", np.int8, (D // GS, GS))
    k_scale = gather("ks", np.float32, (D // GS,))
    k_dq = gather("kdq", np.float32, (D // GS, GS))
    v_q = gather("vq", np.int8, (D // GS, GS))
    v_scale = gather("vs", np.float32, (D // GS,))
    v_dq = gather("vdq", np.float32, (D // GS, GS))
    return k_q, k_scale, v_q, v_scale, k_dq, v_dq


# revision 10
# speedup vs baseline: 1.4778x; 1.0452x over previous
"""Trainium2 Bass kernel for nn_KVCacheQuantizer: int4 group-wise KV-cache
quantize + dequantize round trip.

Full inputs k, v: [4, 32, 4096, 128] fp32. Outputs:
  k_q  [4,32,4096,4,32] int8, k_scale [4,32,4096,4] f32, same for v,
  k_dq / v_dq [4,32,4096,4,32] f32.

Sharded data-parallel over the 128 (batch, head) slabs: core c takes 16
slabs of k and 16 of v; group-wise math (groups of 32 along head_dim) is
fully local.

Per-core pipeline per [128, 2048] fp32 tile (64 groups of 32 per partition):
  a  = absmax over each group          (reduce, abs)
  s  = max(a, 1e-8) * (1/7)
  r  = 1/s                             (DVE iterative reciprocal)
  q  = x * r                           (broadcast per group)
  n  = rne(q)  via +/- 1.5*2^23 magic
  i8 = int8(n)
  dq = n * s
This matches the XLA-on-neuron lowering of the jax reference bit for bit
(verified: the neuron backend lowers x/s to x*reciprocal(s) the same way).
"""

import numpy as np

import concourse.bacc as bacc
import concourse.tile as tile
import concourse.mybir as mybir
from concourse.bass_utils import run_bass_kernel_spmd

B, H, S, D = 4, 32, 4096, 128
GS = 32                      # quantization group size
NCORES = 8
SLABS_PER_CORE = (B * H) // NCORES          # 16 slabs of each tensor
GROUPS_PER_SLAB = S * D // GS               # 16384
T = 64                       # groups per partition per tile
FD = T * GS                  # 2048 fp32 free elems per partition
HALVES = GROUPS_PER_SLAB // (128 * T)       # 2 tiles per slab

EPS = 1e-8
INV7 = float(np.float32(1.0) / np.float32(7.0))
MAGIC = 12582912.0           # 1.5 * 2^23: (q + MAGIC) - MAGIC == rne(q)

# engine knobs (tunable)
FUSE_CAST = True             # fuse round step 2 + int8 cast into one ACT op
DQ_FROM_I8 = True            # dq = int8_tile * s (mixed dtype TT)
CAST_ENGINE = "scalar"       # used when FUSE_CAST is False
DQ_ENGINE = "gpsimd"         # "gpsimd" | "vector"
SCALE_ENGINE = "vector"      # "vector" | "gpsimd"
QMUL_GPSIMD_EVERY = 3        # every Nth tile runs q-mult on gpsimd (0=never)
BUFS = 4
XBUFS = 6
SBUFS = 8

_nc_cache = {}


def _build():
    nc = bacc.Bacc("TRN2", target_bir_lowering=False, debug=False,
                   num_devices=NCORES)
    f32, i8t = mybir.dt.float32, mybir.dt.int8
    n_slabs = SLABS_PER_CORE

    ins = {}
    outs = {}
    for name in ("k", "v"):
        ins[name] = nc.dram_tensor(name, [n_slabs, S, D], f32,
                                   kind="ExternalInput").ap()
        outs[name + "q"] = nc.dram_tensor(name + "q",
                                          [n_slabs, S, D // GS, GS], i8t,
                                          kind="ExternalOutput").ap()
        outs[name + "s"] = nc.dram_tensor(name + "s",
                                          [n_slabs, S, D // GS], f32,
                                          kind="ExternalOutput").ap()
        outs[name + "dq"] = nc.dram_tensor(name + "dq",
                                           [n_slabs, S, D // GS, GS], f32,
                                           kind="ExternalOutput").ap()

    # flat per-(slab, half) tile views: [n_slabs, HALVES, 128, FD]
    def tiled(ap, elems_per_part):
        flat = ap.rearrange("j a b c -> j (a b c)") if ap.ndim == 4 else \
               ap.rearrange("j a b -> j (a b)")
        return flat.rearrange("j (h p f) -> j h p f", h=HALVES, p=128)

    with tile.TileContext(nc) as tc:
        with (
            tc.tile_pool(name="xp", bufs=XBUFS) as xp,
            tc.tile_pool(name="stats", bufs=SBUFS) as statsp,
            tc.tile_pool(name="work", bufs=BUFS) as workp,
            tc.tile_pool(name="outp", bufs=BUFS) as outp,
        ):
            idx = 0
            for name in ("k", "v"):
                x_t = tiled(ins[name], FD)
                q_t = tiled(outs[name + "q"], FD)
                s_t = tiled(outs[name + "s"], T)
                dq_t = tiled(outs[name + "dq"], FD)
                for j in range(SLABS_PER_CORE):
                    for h in range(HALVES):
                        _tile_body(nc, xp, statsp, workp, outp,
                                   x_t[j, h], q_t[j, h], s_t[j, h],
                                   dq_t[j, h], idx)
                        idx += 1
    nc.compile()
    return nc


def _tile_body(nc, xp, statsp, workp, outp, x_in, q_out, s_out, dq_out,
               idx=0):
    f32, i8t = mybir.dt.float32, mybir.dt.int8

    x = xp.tile([128, FD], f32, tag="x")
    nc.sync.dma_start(x[:], x_in)
    xg = x[:].rearrange("p (t e) -> p t e", t=T)

    a = statsp.tile([128, T], f32, tag="a")
    nc.vector.tensor_reduce(a[:], xg, op=mybir.AluOpType.max,
                            axis=mybir.AxisListType.X,
                            apply_absolute_value=True)

    s = statsp.tile([128, T], f32, tag="s")
    scale_eng = nc.gpsimd if SCALE_ENGINE == "gpsimd" else nc.vector
    scale_eng.tensor_scalar(s[:], a[:], float(EPS), INV7,
                            op0=mybir.AluOpType.max,
                            op1=mybir.AluOpType.mult)
    nc.sync.dma_start(s_out, s[:])

    r = statsp.tile([128, T], f32, tag="r")
    nc.vector.reciprocal(r[:], s[:])

    q = workp.tile([128, FD], f32, tag="q")
    qg = q[:].rearrange("p (t e) -> p t e", t=T)
    use_gp = QMUL_GPSIMD_EVERY and (idx % QMUL_GPSIMD_EVERY
                                    == QMUL_GPSIMD_EVERY - 1)
    qmul_eng = nc.gpsimd if use_gp else nc.vector
    qmul_eng.tensor_tensor(qg, xg, r[:].to_broadcast((128, T, GS)),
                           op=mybir.AluOpType.mult)

    t1 = workp.tile([128, FD], f32, tag="t1")
    nc.scalar.activation(t1[:], q[:], mybir.ActivationFunctionType.Copy,
                         bias=MAGIC)

    i8 = outp.tile([128, FD], i8t, tag="i8")
    if FUSE_CAST:
        nc.scalar.activation(i8[:], t1[:], mybir.ActivationFunctionType.Copy,
                             bias=-MAGIC)
        n_ap = i8
    else:
        n = workp.tile([128, FD], f32, tag="n")
        nc.scalar.activation(n[:], t1[:], mybir.ActivationFunctionType.Copy,
                             bias=-MAGIC)
        cast_eng = {"gpsimd": nc.gpsimd, "vector": nc.vector,
                    "scalar": nc.scalar}[CAST_ENGINE]
        if CAST_ENGINE == "scalar":
            cast_eng.activation(i8[:], n[:],
                                mybir.ActivationFunctionType.Copy)
        else:
            cast_eng.tensor_copy(i8[:], n[:])
        n_ap = n
    nc.sync.dma_start(q_out, i8[:])

    dq_src = i8 if DQ_FROM_I8 else n_ap
    dq = outp.tile([128, FD], f32, tag="dq")
    dqg = dq[:].rearrange("p (t e) -> p t e", t=T)
    dq_eng = nc.gpsimd if DQ_ENGINE == "gpsimd" else nc.vector
    dq_eng.tensor_tensor(dqg, dq_src[:].rearrange("p (t e) -> p t e", t=T),
                         s[:].to_broadcast((128, T, GS)),
                         op=mybir.AluOpType.mult)
    nc.sync.dma_start(dq_out, dq[:])


def _get_nc():
    key = (FUSE_CAST, DQ_FROM_I8, CAST_ENGINE, DQ_ENGINE, SCALE_ENGINE,
           QMUL_GPSIMD_EVERY, BUFS, XBUFS, SBUFS, T)
    if key not in _nc_cache:
        _nc_cache[key] = _build()
    return _nc_cache[key]


def kernel(k: np.ndarray, v: np.ndarray, _trace: bool = False):
    nc = _get_nc()
    ksl = np.ascontiguousarray(k).reshape(B * H, S, D)
    vsl = np.ascontiguousarray(v).reshape(B * H, S, D)
    in_maps = []
    for c in range(NCORES):
        sl = slice(c * SLABS_PER_CORE, (c + 1) * SLABS_PER_CORE)
        in_maps.append({"k": ksl[sl], "v": vsl[sl]})
    res = run_bass_kernel_spmd(nc, in_maps, list(range(NCORES)),
                               trace=_trace)
    kernel._last_results = res

    def gather(oname, dtype, tail):
        parts = [res.results[c][oname] for c in range(NCORES)]
        return np.concatenate(parts, axis=0).reshape(B, H, S, *tail) \
                 .astype(dtype, copy=False)

    k_q = gather("kq", np.int8, (D // GS, GS))
    k_scale = gather("ks", np.float32, (D // GS,))
    k_dq = gather("kdq", np.float32, (D // GS, GS))
    v_q = gather("vq", np.int8, (D // GS, GS))
    v_scale = gather("vs", np.float32, (D // GS,))
    v_dq = gather("vdq", np.float32, (D // GS, GS))
    return k_q, k_scale, v_q, v_scale, k_dq, v_dq


# revision 11
# speedup vs baseline: 1.5868x; 1.0737x over previous
"""Trainium2 Bass kernel for nn_KVCacheQuantizer: int4 group-wise KV-cache
quantize + dequantize round trip.

Full inputs k, v: [4, 32, 4096, 128] fp32. Outputs:
  k_q  [4,32,4096,4,32] int8, k_scale [4,32,4096,4] f32, same for v,
  k_dq / v_dq [4,32,4096,4,32] f32.

Sharded data-parallel over the 128 (batch, head) slabs: core c takes 16
slabs of k and 16 of v; group-wise math (groups of 32 along head_dim) is
fully local.

Per-core pipeline per [128, 2048] fp32 tile (64 groups of 32 per partition):
  a  = absmax over each group          (reduce, abs)
  s  = max(a, 1e-8) * (1/7)
  r  = 1/s                             (DVE iterative reciprocal)
  q  = x * r                           (broadcast per group)
  n  = rne(q)  via +/- 1.5*2^23 magic
  i8 = int8(n)
  dq = n * s
This matches the XLA-on-neuron lowering of the jax reference bit for bit
(verified: the neuron backend lowers x/s to x*reciprocal(s) the same way).
"""

import numpy as np

import concourse.bacc as bacc
import concourse.tile as tile
import concourse.mybir as mybir
from concourse.bass_utils import run_bass_kernel_spmd

B, H, S, D = 4, 32, 4096, 128
GS = 32                      # quantization group size
NCORES = 8
SLABS_PER_CORE = (B * H) // NCORES          # 16 slabs of each tensor
GROUPS_PER_SLAB = S * D // GS               # 16384
T = 64                       # groups per partition per tile
FD = T * GS                  # 2048 fp32 free elems per partition
HALVES = GROUPS_PER_SLAB // (128 * T)       # 2 tiles per slab

EPS = 1e-8
INV7 = float(np.float32(1.0) / np.float32(7.0))
MAGIC = 12582912.0           # 1.5 * 2^23: (q + MAGIC) - MAGIC == rne(q)

# engine knobs (tunable)
FUSE_CAST = True             # fuse round step 2 + int8 cast into one ACT op
DQ_FROM_I8 = True            # dq = int8_tile * s (mixed dtype TT)
CAST_ENGINE = "scalar"       # used when FUSE_CAST is False
DQ_ENGINE = "gpsimd"         # "gpsimd" | "vector"
SCALE_ENGINE = "vector"      # "vector" | "gpsimd"
QMUL_GPSIMD_EVERY = 3        # every Nth tile runs q-mult on gpsimd (0=never)
OUT_DMA_ENGINE = "scalar"    # HWDGE ring for output DMAs: "sync" | "scalar"
INPLACE_ROUND = True         # first round op writes q in place (saves t1)
BUFS = 6
XBUFS = 8
SBUFS = 12

_nc_cache = {}


def _build():
    nc = bacc.Bacc("TRN2", target_bir_lowering=False, debug=False,
                   num_devices=NCORES)
    f32, i8t = mybir.dt.float32, mybir.dt.int8
    n_slabs = SLABS_PER_CORE

    ins = {}
    outs = {}
    for name in ("k", "v"):
        ins[name] = nc.dram_tensor(name, [n_slabs, S, D], f32,
                                   kind="ExternalInput").ap()
        outs[name + "q"] = nc.dram_tensor(name + "q",
                                          [n_slabs, S, D // GS, GS], i8t,
                                          kind="ExternalOutput").ap()
        outs[name + "s"] = nc.dram_tensor(name + "s",
                                          [n_slabs, S, D // GS], f32,
                                          kind="ExternalOutput").ap()
        outs[name + "dq"] = nc.dram_tensor(name + "dq",
                                           [n_slabs, S, D // GS, GS], f32,
                                           kind="ExternalOutput").ap()

    # flat per-(slab, half) tile views: [n_slabs, HALVES, 128, FD]
    def tiled(ap, elems_per_part):
        flat = ap.rearrange("j a b c -> j (a b c)") if ap.ndim == 4 else \
               ap.rearrange("j a b -> j (a b)")
        return flat.rearrange("j (h p f) -> j h p f", h=HALVES, p=128)

    with tile.TileContext(nc) as tc:
        with (
            tc.tile_pool(name="xp", bufs=XBUFS) as xp,
            tc.tile_pool(name="stats", bufs=SBUFS) as statsp,
            tc.tile_pool(name="work", bufs=BUFS) as workp,
            tc.tile_pool(name="outp", bufs=BUFS) as outp,
        ):
            idx = 0
            for name in ("k", "v"):
                x_t = tiled(ins[name], FD)
                q_t = tiled(outs[name + "q"], FD)
                s_t = tiled(outs[name + "s"], T)
                dq_t = tiled(outs[name + "dq"], FD)
                for j in range(SLABS_PER_CORE):
                    for h in range(HALVES):
                        _tile_body(nc, xp, statsp, workp, outp,
                                   x_t[j, h], q_t[j, h], s_t[j, h],
                                   dq_t[j, h], idx)
                        idx += 1
    nc.compile()
    return nc


def _tile_body(nc, xp, statsp, workp, outp, x_in, q_out, s_out, dq_out,
               idx=0):
    f32, i8t = mybir.dt.float32, mybir.dt.int8

    x = xp.tile([128, FD], f32, tag="x")
    nc.sync.dma_start(x[:], x_in)
    xg = x[:].rearrange("p (t e) -> p t e", t=T)

    a = statsp.tile([128, T], f32, tag="a")
    nc.vector.tensor_reduce(a[:], xg, op=mybir.AluOpType.max,
                            axis=mybir.AxisListType.X,
                            apply_absolute_value=True)

    s = statsp.tile([128, T], f32, tag="s")
    scale_eng = nc.gpsimd if SCALE_ENGINE == "gpsimd" else nc.vector
    scale_eng.tensor_scalar(s[:], a[:], float(EPS), INV7,
                            op0=mybir.AluOpType.max,
                            op1=mybir.AluOpType.mult)
    nc.sync.dma_start(s_out, s[:])

    r = statsp.tile([128, T], f32, tag="r")
    nc.vector.reciprocal(r[:], s[:])

    q = workp.tile([128, FD], f32, tag="q")
    qg = q[:].rearrange("p (t e) -> p t e", t=T)
    use_gp = QMUL_GPSIMD_EVERY and (idx % QMUL_GPSIMD_EVERY
                                    == QMUL_GPSIMD_EVERY - 1)
    qmul_eng = nc.gpsimd if use_gp else nc.vector
    qmul_eng.tensor_tensor(qg, xg, r[:].to_broadcast((128, T, GS)),
                           op=mybir.AluOpType.mult)

    if INPLACE_ROUND:
        t1 = q
    else:
        t1 = workp.tile([128, FD], f32, tag="t1")
    nc.scalar.activation(t1[:], q[:], mybir.ActivationFunctionType.Copy,
                         bias=MAGIC)

    out_dma = nc.scalar if OUT_DMA_ENGINE == "scalar" else nc.sync
    i8 = outp.tile([128, FD], i8t, tag="i8")
    nc.scalar.activation(i8[:], t1[:], mybir.ActivationFunctionType.Copy,
                         bias=-MAGIC)
    out_dma.dma_start(q_out, i8[:])

    dq = outp.tile([128, FD], f32, tag="dq")
    dqg = dq[:].rearrange("p (t e) -> p t e", t=T)
    dq_eng = nc.gpsimd if DQ_ENGINE == "gpsimd" else nc.vector
    dq_eng.tensor_tensor(dqg, i8[:].rearrange("p (t e) -> p t e", t=T),
                         s[:].to_broadcast((128, T, GS)),
                         op=mybir.AluOpType.mult)
    out_dma.dma_start(dq_out, dq[:])


def _get_nc():
    key = (FUSE_CAST, DQ_FROM_I8, DQ_ENGINE, SCALE_ENGINE,
           QMUL_GPSIMD_EVERY, OUT_DMA_ENGINE, INPLACE_ROUND,
           BUFS, XBUFS, SBUFS, T)
    if key not in _nc_cache:
        _nc_cache[key] = _build()
    return _nc_cache[key]


def kernel(k: np.ndarray, v: np.ndarray, _trace: bool = False):
    nc = _get_nc()
    ksl = np.ascontiguousarray(k).reshape(B * H, S, D)
    vsl = np.ascontiguousarray(v).reshape(B * H, S, D)
    in_maps = []
    for c in range(NCORES):
        sl = slice(c * SLABS_PER_CORE, (c + 1) * SLABS_PER_CORE)
        in_maps.append({"k": ksl[sl], "v": vsl[sl]})
    res = run_bass_kernel_spmd(nc, in_maps, list(range(NCORES)),
                               trace=_trace)
    kernel._last_results = res

    def gather(oname, dtype, tail):
        parts = [res.results[c][oname] for c in range(NCORES)]
        return np.concatenate(parts, axis=0).reshape(B, H, S, *tail) \
                 .astype(dtype, copy=False)

    k_q = gather("kq", np.int8, (D // GS, GS))
    k_scale = gather("ks", np.float32, (D // GS,))
    k_dq = gather("kdq", np.float32, (D // GS, GS))
    v_q = gather("vq", np.int8, (D // GS, GS))
    v_scale = gather("vs", np.float32, (D // GS,))
    v_dq = gather("vdq", np.float32, (D // GS, GS))
    return k_q, k_scale, v_q, v_scale, k_dq, v_dq


# revision 12
# speedup vs baseline: 1.7135x; 1.0799x over previous
"""Trainium2 Bass kernel for nn_KVCacheQuantizer: int4 group-wise KV-cache
quantize + dequantize round trip.

Full inputs k, v: [4, 32, 4096, 128] fp32. Outputs:
  k_q  [4,32,4096,4,32] int8, k_scale [4,32,4096,4] f32, same for v,
  k_dq / v_dq [4,32,4096,4,32] f32.

Sharded data-parallel over the 128 (batch, head) slabs: core c takes 16
slabs of k and 16 of v; group-wise math (groups of 32 along head_dim) is
fully local.

Per-core pipeline per [128, 2048] fp32 tile (64 groups of 32 per partition):
  a  = absmax over each group          (reduce, abs)
  s  = max(a, 1e-8) * (1/7)
  r  = 1/s                             (DVE iterative reciprocal)
  q  = x * r                           (broadcast per group)
  n  = rne(q)  via +/- 1.5*2^23 magic
  i8 = int8(n)
  dq = n * s
This matches the XLA-on-neuron lowering of the jax reference bit for bit
(verified: the neuron backend lowers x/s to x*reciprocal(s) the same way).
"""

import numpy as np

import concourse.bacc as bacc
import concourse.tile as tile
import concourse.mybir as mybir
from concourse.bass_utils import run_bass_kernel_spmd

B, H, S, D = 4, 32, 4096, 128
GS = 32                      # quantization group size
NCORES = 8
SLABS_PER_CORE = (B * H) // NCORES          # 16 slabs of each tensor
GROUPS_PER_SLAB = S * D // GS               # 16384
T = 128                      # groups per partition per tile
FD = T * GS                  # 2048 fp32 free elems per partition
HALVES = GROUPS_PER_SLAB // (128 * T)       # 2 tiles per slab

EPS = 1e-8
INV7 = float(np.float32(1.0) / np.float32(7.0))
MAGIC = 12582912.0           # 1.5 * 2^23: (q + MAGIC) - MAGIC == rne(q)

# engine knobs (tunable)
FUSE_CAST = True             # fuse round step 2 + int8 cast into one ACT op
DQ_FROM_I8 = True            # dq = int8_tile * s (mixed dtype TT)
CAST_ENGINE = "scalar"       # used when FUSE_CAST is False
DQ_ENGINE = "gpsimd"         # "gpsimd" | "vector"
SCALE_ENGINE = "vector"      # "vector" | "gpsimd"
QMUL_GPSIMD_EVERY = 3        # every Nth tile runs q-mult on gpsimd (0=never)
OUT_DMA_ENGINE = "scalar"    # HWDGE ring for output DMAs: "sync" | "scalar"
INPLACE_ROUND = True         # first round op writes q in place (saves t1)
BUFS = 3
XBUFS = 4
SBUFS = 8

_nc_cache = {}


def _build():
    nc = bacc.Bacc("TRN2", target_bir_lowering=False, debug=False,
                   num_devices=NCORES)
    f32, i8t = mybir.dt.float32, mybir.dt.int8
    n_slabs = SLABS_PER_CORE

    ins = {}
    outs = {}
    for name in ("k", "v"):
        ins[name] = nc.dram_tensor(name, [n_slabs, S, D], f32,
                                   kind="ExternalInput").ap()
        outs[name + "q"] = nc.dram_tensor(name + "q",
                                          [n_slabs, S, D // GS, GS], i8t,
                                          kind="ExternalOutput").ap()
        outs[name + "s"] = nc.dram_tensor(name + "s",
                                          [n_slabs, S, D // GS], f32,
                                          kind="ExternalOutput").ap()
        outs[name + "dq"] = nc.dram_tensor(name + "dq",
                                           [n_slabs, S, D // GS, GS], f32,
                                           kind="ExternalOutput").ap()

    # flat per-(slab, half) tile views: [n_slabs, HALVES, 128, FD]
    def tiled(ap, elems_per_part):
        flat = ap.rearrange("j a b c -> j (a b c)") if ap.ndim == 4 else \
               ap.rearrange("j a b -> j (a b)")
        return flat.rearrange("j (h p f) -> j h p f", h=HALVES, p=128)

    with tile.TileContext(nc) as tc:
        with (
            tc.tile_pool(name="xp", bufs=XBUFS) as xp,
            tc.tile_pool(name="stats", bufs=SBUFS) as statsp,
            tc.tile_pool(name="work", bufs=BUFS) as workp,
            tc.tile_pool(name="outp", bufs=BUFS) as outp,
        ):
            idx = 0
            for name in ("k", "v"):
                x_t = tiled(ins[name], FD)
                q_t = tiled(outs[name + "q"], FD)
                s_t = tiled(outs[name + "s"], T)
                dq_t = tiled(outs[name + "dq"], FD)
                for j in range(SLABS_PER_CORE):
                    for h in range(HALVES):
                        _tile_body(nc, xp, statsp, workp, outp,
                                   x_t[j, h], q_t[j, h], s_t[j, h],
                                   dq_t[j, h], idx)
                        idx += 1
    nc.compile()
    return nc


def _tile_body(nc, xp, statsp, workp, outp, x_in, q_out, s_out, dq_out,
               idx=0):
    f32, i8t = mybir.dt.float32, mybir.dt.int8

    x = xp.tile([128, FD], f32, tag="x")
    nc.sync.dma_start(x[:], x_in)
    xg = x[:].rearrange("p (t e) -> p t e", t=T)

    a = statsp.tile([128, T], f32, tag="a")
    nc.vector.tensor_reduce(a[:], xg, op=mybir.AluOpType.max,
                            axis=mybir.AxisListType.X,
                            apply_absolute_value=True)

    s = statsp.tile([128, T], f32, tag="s")
    scale_eng = nc.gpsimd if SCALE_ENGINE == "gpsimd" else nc.vector
    scale_eng.tensor_scalar(s[:], a[:], float(EPS), INV7,
                            op0=mybir.AluOpType.max,
                            op1=mybir.AluOpType.mult)
    nc.sync.dma_start(s_out, s[:])

    r = statsp.tile([128, T], f32, tag="r")
    nc.vector.reciprocal(r[:], s[:])

    q = workp.tile([128, FD], f32, tag="q")
    qg = q[:].rearrange("p (t e) -> p t e", t=T)
    use_gp = QMUL_GPSIMD_EVERY and (idx % QMUL_GPSIMD_EVERY
                                    == QMUL_GPSIMD_EVERY - 1)
    qmul_eng = nc.gpsimd if use_gp else nc.vector
    qmul_eng.tensor_tensor(qg, xg, r[:].to_broadcast((128, T, GS)),
                           op=mybir.AluOpType.mult)

    if INPLACE_ROUND:
        t1 = q
    else:
        t1 = workp.tile([128, FD], f32, tag="t1")
    nc.scalar.activation(t1[:], q[:], mybir.ActivationFunctionType.Copy,
                         bias=MAGIC)

    out_dma = nc.scalar if OUT_DMA_ENGINE == "scalar" else nc.sync
    i8 = outp.tile([128, FD], i8t, tag="i8")
    nc.scalar.activation(i8[:], t1[:], mybir.ActivationFunctionType.Copy,
                         bias=-MAGIC)
    out_dma.dma_start(q_out, i8[:])

    dq = outp.tile([128, FD], f32, tag="dq")
    dqg = dq[:].rearrange("p (t e) -> p t e", t=T)
    dq_eng = nc.gpsimd if DQ_ENGINE == "gpsimd" else nc.vector
    dq_eng.tensor_tensor(dqg, i8[:].rearrange("p (t e) -> p t e", t=T),
                         s[:].to_broadcast((128, T, GS)),
                         op=mybir.AluOpType.mult)
    out_dma.dma_start(dq_out, dq[:])


def _get_nc():
    key = (FUSE_CAST, DQ_FROM_I8, DQ_ENGINE, SCALE_ENGINE,
           QMUL_GPSIMD_EVERY, OUT_DMA_ENGINE, INPLACE_ROUND,
           BUFS, XBUFS, SBUFS, T)
    if key not in _nc_cache:
        _nc_cache[key] = _build()
    return _nc_cache[key]


def kernel(k: np.ndarray, v: np.ndarray, _trace: bool = False):
    nc = _get_nc()
    ksl = np.ascontiguousarray(k).reshape(B * H, S, D)
    vsl = np.ascontiguousarray(v).reshape(B * H, S, D)
    in_maps = []
    for c in range(NCORES):
        sl = slice(c * SLABS_PER_CORE, (c + 1) * SLABS_PER_CORE)
        in_maps.append({"k": ksl[sl], "v": vsl[sl]})
    res = run_bass_kernel_spmd(nc, in_maps, list(range(NCORES)),
                               trace=_trace)
    kernel._last_results = res

    def gather(oname, dtype, tail):
        parts = [res.results[c][oname] for c in range(NCORES)]
        return np.concatenate(parts, axis=0).reshape(B, H, S, *tail) \
                 .astype(dtype, copy=False)

    k_q = gather("kq", np.int8, (D // GS, GS))
    k_scale = gather("ks", np.float32, (D // GS,))
    k_dq = gather("kdq", np.float32, (D // GS, GS))
    v_q = gather("vq", np.int8, (D // GS, GS))
    v_scale = gather("vs", np.float32, (D // GS,))
    v_dq = gather("vdq", np.float32, (D // GS, GS))
    return k_q, k_scale, v_q, v_scale, k_dq, v_dq


# revision 13
# speedup vs baseline: 1.8341x; 1.0703x over previous
"""Trainium2 Bass kernel for nn_KVCacheQuantizer: int4 group-wise KV-cache
quantize + dequantize round trip.

Full inputs k, v: [4, 32, 4096, 128] fp32. Outputs:
  k_q  [4,32,4096,4,32] int8, k_scale [4,32,4096,4] f32, same for v,
  k_dq / v_dq [4,32,4096,4,32] f32.

Sharded data-parallel over the 128 (batch, head) slabs: core c takes 16
slabs of k and 16 of v; group-wise math (groups of 32 along head_dim) is
fully local.

Per-core pipeline per [128, 2048] fp32 tile (64 groups of 32 per partition):
  a  = absmax over each group          (reduce, abs)
  s  = max(a, 1e-8) * (1/7)
  r  = 1/s                             (DVE iterative reciprocal)
  q  = x * r                           (broadcast per group)
  n  = rne(q)  via +/- 1.5*2^23 magic
  i8 = int8(n)
  dq = n * s
This matches the XLA-on-neuron lowering of the jax reference bit for bit
(verified: the neuron backend lowers x/s to x*reciprocal(s) the same way).
"""

import numpy as np

import concourse.bacc as bacc
import concourse.tile as tile
import concourse.mybir as mybir
from concourse.bass_utils import run_bass_kernel_spmd

B, H, S, D = 4, 32, 4096, 128
GS = 32                      # quantization group size
NCORES = 8
SLABS_PER_CORE = (B * H) // NCORES          # 16 slabs of each tensor
GROUPS_PER_SLAB = S * D // GS               # 16384
T = 128                      # groups per partition per tile
FD = T * GS                  # 2048 fp32 free elems per partition
HALVES = GROUPS_PER_SLAB // (128 * T)       # 2 tiles per slab

EPS = 1e-8
INV7 = float(np.float32(1.0) / np.float32(7.0))
MAGIC = 12582912.0           # 1.5 * 2^23: (q + MAGIC) - MAGIC == rne(q)

# engine knobs (tunable)
FUSE_CAST = True             # fuse round step 2 + int8 cast into one ACT op
DQ_FROM_I8 = True            # dq = int8_tile * s (mixed dtype TT)
CAST_ENGINE = "scalar"       # used when FUSE_CAST is False
DQ_ENGINE = "gpsimd"         # "gpsimd" | "vector"
SCALE_ENGINE = "vector"      # "vector" | "gpsimd"
QMUL_GPSIMD_EVERY = 0        # every Nth tile runs q-mult on gpsimd (0=never)
OUT_DMA_ENGINE = "scalar"    # HWDGE ring for output DMAs: "sync" | "scalar"
INPLACE_ROUND = True         # first round op writes q in place (saves t1)
BUFS = 3
XBUFS = 4
SBUFS = 8

_nc_cache = {}


def _build():
    nc = bacc.Bacc("TRN2", target_bir_lowering=False, debug=False,
                   num_devices=NCORES)
    f32, i8t = mybir.dt.float32, mybir.dt.int8
    n_slabs = SLABS_PER_CORE

    ins = {}
    outs = {}
    for name in ("k", "v"):
        ins[name] = nc.dram_tensor(name, [n_slabs, S, D], f32,
                                   kind="ExternalInput").ap()
        outs[name + "q"] = nc.dram_tensor(name + "q",
                                          [n_slabs, S, D // GS, GS], i8t,
                                          kind="ExternalOutput").ap()
        outs[name + "s"] = nc.dram_tensor(name + "s",
                                          [n_slabs, S, D // GS], f32,
                                          kind="ExternalOutput").ap()
        outs[name + "dq"] = nc.dram_tensor(name + "dq",
                                           [n_slabs, S, D // GS, GS], f32,
                                           kind="ExternalOutput").ap()

    # flat per-(slab, half) tile views: [n_slabs, HALVES, 128, FD]
    def tiled(ap, elems_per_part):
        flat = ap.rearrange("j a b c -> j (a b c)") if ap.ndim == 4 else \
               ap.rearrange("j a b -> j (a b)")
        return flat.rearrange("j (h p f) -> j h p f", h=HALVES, p=128)

    with tile.TileContext(nc) as tc:
        with (
            tc.tile_pool(name="xp", bufs=XBUFS) as xp,
            tc.tile_pool(name="stats", bufs=SBUFS) as statsp,
            tc.tile_pool(name="work", bufs=BUFS) as workp,
            tc.tile_pool(name="outp", bufs=BUFS) as outp,
        ):
            idx = 0
            for name in ("k", "v"):
                x_t = tiled(ins[name], FD)
                q_t = tiled(outs[name + "q"], FD)
                s_t = tiled(outs[name + "s"], T)
                dq_t = tiled(outs[name + "dq"], FD)
                for j in range(SLABS_PER_CORE):
                    for h in range(HALVES):
                        _tile_body(nc, xp, statsp, workp, outp,
                                   x_t[j, h], q_t[j, h], s_t[j, h],
                                   dq_t[j, h], idx)
                        idx += 1
    nc.compile()
    return nc


def _tile_body(nc, xp, statsp, workp, outp, x_in, q_out, s_out, dq_out,
               idx=0):
    f32, i8t = mybir.dt.float32, mybir.dt.int8

    x = xp.tile([128, FD], f32, tag="x")
    nc.sync.dma_start(x[:], x_in)
    xg = x[:].rearrange("p (t e) -> p t e", t=T)

    a = statsp.tile([128, T], f32, tag="a")
    nc.vector.tensor_reduce(a[:], xg, op=mybir.AluOpType.max,
                            axis=mybir.AxisListType.X,
                            apply_absolute_value=True)

    s = statsp.tile([128, T], f32, tag="s")
    scale_eng = nc.gpsimd if SCALE_ENGINE == "gpsimd" else nc.vector
    scale_eng.tensor_scalar(s[:], a[:], float(EPS), INV7,
                            op0=mybir.AluOpType.max,
                            op1=mybir.AluOpType.mult)
    nc.sync.dma_start(s_out, s[:])

    r = statsp.tile([128, T], f32, tag="r")
    nc.vector.reciprocal(r[:], s[:])

    q = workp.tile([128, FD], f32, tag="q")
    qg = q[:].rearrange("p (t e) -> p t e", t=T)
    use_gp = QMUL_GPSIMD_EVERY and (idx % QMUL_GPSIMD_EVERY
                                    == QMUL_GPSIMD_EVERY - 1)
    qmul_eng = nc.gpsimd if use_gp else nc.vector
    qmul_eng.tensor_tensor(qg, xg, r[:].to_broadcast((128, T, GS)),
                           op=mybir.AluOpType.mult)

    if INPLACE_ROUND:
        t1 = q
    else:
        t1 = workp.tile([128, FD], f32, tag="t1")
    nc.scalar.activation(t1[:], q[:], mybir.ActivationFunctionType.Copy,
                         bias=MAGIC)

    out_dma = nc.scalar if OUT_DMA_ENGINE == "scalar" else nc.sync
    i8 = outp.tile([128, FD], i8t, tag="i8")
    nc.scalar.activation(i8[:], t1[:], mybir.ActivationFunctionType.Copy,
                         bias=-MAGIC)
    out_dma.dma_start(q_out, i8[:])

    dq = outp.tile([128, FD], f32, tag="dq")
    dqg = dq[:].rearrange("p (t e) -> p t e", t=T)
    dq_eng = nc.gpsimd if DQ_ENGINE == "gpsimd" else nc.vector
    dq_eng.tensor_tensor(dqg, i8[:].rearrange("p (t e) -> p t e", t=T),
                         s[:].to_broadcast((128, T, GS)),
                         op=mybir.AluOpType.mult)
    out_dma.dma_start(dq_out, dq[:])


def _get_nc():
    key = (FUSE_CAST, DQ_FROM_I8, DQ_ENGINE, SCALE_ENGINE,
           QMUL_GPSIMD_EVERY, OUT_DMA_ENGINE, INPLACE_ROUND,
           BUFS, XBUFS, SBUFS, T)
    if key not in _nc_cache:
        _nc_cache[key] = _build()
    return _nc_cache[key]


def kernel(k: np.ndarray, v: np.ndarray, _trace: bool = False):
    nc = _get_nc()
    ksl = np.ascontiguousarray(k).reshape(B * H, S, D)
    vsl = np.ascontiguousarray(v).reshape(B * H, S, D)
    in_maps = []
    for c in range(NCORES):
        sl = slice(c * SLABS_PER_CORE, (c + 1) * SLABS_PER_CORE)
        in_maps.append({"k": ksl[sl], "v": vsl[sl]})
    res = run_bass_kernel_spmd(nc, in_maps, list(range(NCORES)),
                               trace=_trace)
    kernel._last_results = res

    def gather(oname, dtype, tail):
        parts = [res.results[c][oname] for c in range(NCORES)]
        return np.concatenate(parts, axis=0).reshape(B, H, S, *tail) \
                 .astype(dtype, copy=False)

    k_q = gather("kq", np.int8, (D // GS, GS))
    k_scale = gather("ks", np.float32, (D // GS,))
    k_dq = gather("kdq", np.float32, (D // GS, GS))
    v_q = gather("vq", np.int8, (D // GS, GS))
    v_scale = gather("vs", np.float32, (D // GS,))
    v_dq = gather("vdq", np.float32, (D // GS, GS))
    return k_q, k_scale, v_q, v_scale, k_dq, v_dq
